# revision 2
# baseline (speedup 1.0000x reference)
"""Trainium2 Bass kernel for a 2-layer LSTM regressor.

Model: x[1024,512,64] -> LSTM(64->256) -> LSTM(256->256) -> tanh(fc(h_last)) -> [1024]

Strategy (data-parallel over batch, 8 cores, 128 rows/core):
  - All matmul inputs in bf16; PSUM accumulation in fp32.
  - sigmoid folded into tanh: sigmoid(z) = 0.5*(1+tanh(z/2)); the 0.5 input
    prescale is folded into the i/f/o rows of all weight matrices, so ONE
    tanh activation covers all 4 gate blocks.
  - State rescaling: store h2 := 2h and C := 2c. Then with tg* = tanh(gates):
        u = (tf+1)*C          (one scalar_tensor_tensor)
        v = (ti+1)*tg         (one scalar_tensor_tensor)
        C_new = 0.5*u + v     (one scalar_tensor_tensor)
        tanh_c = tanh(0.5*C)  (ACT with free scale=0.5)
        h2 = (to+1)*tanh_c    (one scalar_tensor_tensor)
    and all h2-consuming weight matrices are prescaled by 0.5.
  - Recurrent matmuls keep batch on PSUM partitions: gates[B=128, 4H] with
    stationary lhsT = transposed state hT [H, B]; h2 is transposed each step
    on the PE (2x 128x128 transposes/layer) and copied PSUM->SBUF by DVE.
  - Biases: layer-0 bias rides in an appended ones-row of xT (K=65);
    layer-1 bias is a rank-1 matmul (ones[1,128] x bias[1,4H]).
  - x is pre-transposed host-side to [65, T, 128] per core and fully resident
    in SBUF (bf16), so the steady state does no DMA at all.
"""

import numpy as np
import ml_dtypes

import concourse.bass as bass
import concourse.mybir as mybir
import concourse.tile as tile
from concourse import bacc
from concourse.bass_utils import run_bass_kernel_spmd

B, T, D, H = 1024, 512, 64, 256
NCORES = 8
BLOC = B // NCORES  # 128
G4 = 4 * H          # 1024

BF16 = mybir.dt.bfloat16
F32 = mybir.dt.float32
AF = mybir.ActivationFunctionType
OP = mybir.AluOpType
NPBF = ml_dtypes.bfloat16

# Module-level knobs (test.py may override before calling kernel()).
T_STEPS = T
TRACE = False
TRACE_KW = {}

_build_cache = {}


def _build(t_steps):
    nc = bacc.Bacc(
        "TRN2",
        target_bir_lowering=False,
        debug=False,
        enable_asserts=False,
        num_devices=NCORES,
    )

    xt = nc.dram_tensor("xt", [D + 1, t_steps, BLOC], BF16, kind="ExternalInput").ap()
    rih0 = nc.dram_tensor("rih0", [D + 1, G4], BF16, kind="ExternalInput").ap()
    rhh0 = nc.dram_tensor("rhh0", [128, 2 * G4], BF16, kind="ExternalInput").ap()
    rih1 = nc.dram_tensor("rih1", [128, 2 * G4], BF16, kind="ExternalInput").ap()
    rhh1 = nc.dram_tensor("rhh1", [128, 2 * G4], BF16, kind="ExternalInput").ap()
    rb1 = nc.dram_tensor("rb1", [1, G4], BF16, kind="ExternalInput").ap()
    onesd = nc.dram_tensor("ones", [1, BLOC], BF16, kind="ExternalInput").ap()
    identd = nc.dram_tensor("ident", [128, 128], BF16, kind="ExternalInput").ap()
    fcwd = nc.dram_tensor("fcw", [128, 2], BF16, kind="ExternalInput").ap()
    fcbd = nc.dram_tensor("fcb", [128, 1], F32, kind="ExternalInput").ap()
    outd = nc.dram_tensor("out", [BLOC, 1], F32, kind="ExternalOutput").ap()

    with tile.TileContext(nc) as tc:
        with (
            tc.tile_pool(name="const", bufs=1) as cp,
            tc.tile_pool(name="tg", bufs=2) as tgp,
            tc.tile_pool(name="uvp", bufs=2) as uvp,
            tc.tile_pool(name="cst", bufs=2) as cstp,
            tc.tile_pool(name="hT", bufs=3) as hTp,
            tc.tile_pool(name="gpsum", bufs=1, space="PSUM") as gp,
            tc.tile_pool(name="hpsum", bufs=2, space="PSUM") as hp,
        ):
            # ---- constants / inputs resident in SBUF ----
            xt_sb = cp.tile([D + 1, t_steps, BLOC], BF16)
            chunk = 64
            for c0 in range(0, t_steps, chunk):
                c1 = min(c0 + chunk, t_steps)
                nc.sync.dma_start(out=xt_sb[:, c0:c1, :], in_=xt[:, c0:c1, :])
            rih0_sb = cp.tile([D + 1, G4], BF16)
            nc.sync.dma_start(out=rih0_sb, in_=rih0)
            rhh0_sb = cp.tile([128, 2 * G4], BF16)
            nc.sync.dma_start(out=rhh0_sb, in_=rhh0)
            rih1_sb = cp.tile([128, 2 * G4], BF16)
            nc.sync.dma_start(out=rih1_sb, in_=rih1)
            rhh1_sb = cp.tile([128, 2 * G4], BF16)
            nc.sync.dma_start(out=rhh1_sb, in_=rhh1)
            rb1_sb = cp.tile([1, G4], BF16)
            nc.sync.dma_start(out=rb1_sb, in_=rb1)
            ones_sb = cp.tile([1, BLOC], BF16)
            nc.sync.dma_start(out=ones_sb, in_=onesd)
            ident_sb = cp.tile([128, 128], BF16)
            nc.sync.dma_start(out=ident_sb, in_=identd)
            fcw_sb = cp.tile([128, 2], BF16)
            nc.sync.dma_start(out=fcw_sb, in_=fcwd)
            fcb_sb = cp.tile([128, 1], F32)
            nc.sync.dma_start(out=fcb_sb, in_=fcbd)

            # ---- initial state ----
            h0T = hTp.tile([128, 2 * BLOC], BF16, tag="h0T")
            h1T = hTp.tile([128, 2 * BLOC], BF16, tag="h1T")
            C0 = cstp.tile([128, H], F32, tag="C0")
            C1 = cstp.tile([128, H], F32, tag="C1")
            nc.vector.memset(h0T, 0.0)
            nc.vector.memset(h1T, 0.0)
            nc.vector.memset(C0, 0.0)
            nc.vector.memset(C1, 0.0)

            def half_step(layer, t, hT_prev, C_prev, h0T_new):
                """Emit one LSTM cell update for one layer at step t.
                layer 0: input contribution from xt (K=65, carries bias).
                layer 1: input contribution from fresh h0T_new + rank-1 bias.
                Returns (hT_new, C_new)."""
                G = gp.tile([128, G4], F32, tag=f"G{layer}")
                rhh = rhh0_sb if layer == 0 else rhh1_sb
                for nb in range(2):  # the two PSUM banks / N-halves
                    n0, n1 = nb * 512, (nb + 1) * 512
                    if layer == 0:
                        nc.tensor.matmul(
                            G[:, n0:n1], xt_sb[:, t, :], rih0_sb[:, n0:n1],
                            start=True, stop=False,
                        )
                    else:
                        nc.tensor.matmul(
                            G[:, n0:n1], ones_sb, rb1_sb[:, n0:n1],
                            start=True, stop=False,
                        )
                        for k in range(2):
                            nc.tensor.matmul(
                                G[:, n0:n1],
                                h0T_new[:, k * BLOC:(k + 1) * BLOC],
                                rih1_sb[:, k * G4 + n0:k * G4 + n1],
                                start=False, stop=False,
                            )
                    for k in range(2):
                        nc.tensor.matmul(
                            G[:, n0:n1],
                            hT_prev[:, k * BLOC:(k + 1) * BLOC],
                            rhh[:, k * G4 + n0:k * G4 + n1],
                            start=False, stop=(k == 1),
                        )

                tg = tgp.tile([128, G4], BF16, tag=f"tg{layer}")
                nc.scalar.activation(tg[:, 0:512], G[:, 0:512], AF.Tanh)
                nc.scalar.activation(tg[:, 512:1024], G[:, 512:1024], AF.Tanh)

                # u = (tf+1)*C ; v = (ti+1)*tg
                u = uvp.tile([128, H], F32, tag=f"u{layer}")
                nc.vector.scalar_tensor_tensor(
                    u, tg[:, H:2 * H], 1.0, C_prev, op0=OP.add, op1=OP.mult
                )
                v = uvp.tile([128, H], BF16, tag=f"v{layer}")
                nc.vector.scalar_tensor_tensor(
                    v, tg[:, 0:H], 1.0, tg[:, 2 * H:3 * H], op0=OP.add, op1=OP.mult
                )
                C_new = cstp.tile([128, H], F32, tag=f"C{layer}")
                nc.vector.scalar_tensor_tensor(
                    C_new, u, 0.5, v, op0=OP.mult, op1=OP.add
                )
                tc_ = uvp.tile([128, H], BF16, tag=f"tc{layer}")
                nc.scalar.activation(tc_, C_new, AF.Tanh, scale=0.5)
                h2 = uvp.tile([128, H], BF16, tag=f"h2{layer}")
                nc.vector.scalar_tensor_tensor(
                    h2, tg[:, 3 * H:4 * H], 1.0, tc_, op0=OP.add, op1=OP.mult
                )
                # transpose h2 [128,256] -> hT [2 chunks of 128x128]
                hps = hp.tile([128, 2 * BLOC], BF16, tag="hps")
                nc.tensor.transpose(hps[:, 0:BLOC], h2[:, 0:BLOC], ident_sb)
                nc.tensor.transpose(hps[:, BLOC:2 * BLOC], h2[:, BLOC:2 * BLOC], ident_sb)
                hT_new = hTp.tile([128, 2 * BLOC], BF16, tag=f"h{layer}T")
                nc.vector.tensor_copy(hT_new, hps)
                return hT_new, C_new

            for t in range(t_steps):
                h0T, C0 = half_step(0, t, h0T, C0, None)
                h1T, C1 = half_step(1, t, h1T, C1, h0T)

            # ---- final FC head: out = tanh(h1 @ (fc_w.T) + fc_b) ----
            Gfc = gp.tile([128, G4], F32, tag="G0")
            nc.tensor.matmul(Gfc[:, 0:1], h1T[:, 0:BLOC], fcw_sb[:, 0:1],
                             start=True, stop=False)
            nc.tensor.matmul(Gfc[:, 0:1], h1T[:, BLOC:2 * BLOC], fcw_sb[:, 1:2],
                             start=False, stop=True)
            out_sb = cp.tile([128, 1], F32)
            nc.scalar.activation(out_sb, Gfc[:, 0:1], AF.Tanh, bias=fcb_sb)
            nc.sync.dma_start(out=outd, in_=out_sb)

    nc.compile()
    return nc


def _get_nc(t_steps):
    if t_steps not in _build_cache:
        _build_cache[t_steps] = _build(t_steps)
    return _build_cache[t_steps]


def _prep_inputs(x, W_ih0, W_hh0, b_ih0, b_hh0, W_ih1, W_hh1, b_ih1, b_hh1,
                 fc_w, fc_b, t_steps):
    f32 = np.float32
    gs = np.ones(G4, f32)
    gs[: 2 * H] = 0.5
    gs[3 * H:] = 0.5  # i, f, o prescaled by 0.5 (tanh->sigmoid); g unscaled

    def chunks2(a):  # [256, G4] -> [128, 2*G4] with chunk k at cols [k*G4:(k+1)*G4]
        return np.concatenate([a[0:128, :], a[128:256, :]], axis=1)

    rih0 = (np.concatenate([W_ih0.T.astype(f32), (b_ih0 + b_hh0)[None, :].astype(f32)], 0) * gs)
    rhh0 = chunks2(W_hh0.T.astype(f32) * 0.5 * gs)
    rih1 = chunks2(W_ih1.T.astype(f32) * 0.5 * gs)
    rhh1 = chunks2(W_hh1.T.astype(f32) * 0.5 * gs)
    rb1 = ((b_ih1 + b_hh1).astype(f32) * gs)[None, :]
    fcw = np.empty((128, 2), f32)
    fcw[:, 0] = fc_w[0, 0:128] * 0.5
    fcw[:, 1] = fc_w[0, 128:256] * 0.5

    shared = {
        "rih0": rih0.astype(NPBF),
        "rhh0": rhh0.astype(NPBF),
        "rih1": rih1.astype(NPBF),
        "rhh1": rhh1.astype(NPBF),
        "rb1": rb1.astype(NPBF),
        "ones": np.ones((1, BLOC), NPBF),
        "ident": np.eye(128, dtype=NPBF),
        "fcw": fcw.astype(NPBF),
        "fcb": np.full((128, 1), np.float32(fc_b[0]), f32),
    }

    in_maps = []
    for c in range(NCORES):
        xc = np.asarray(x[c * BLOC:(c + 1) * BLOC, :t_steps, :], f32)  # [128, t, 64]
        xb = np.concatenate([xc, np.ones((BLOC, t_steps, 1), f32)], 2)  # [128, t, 65]
        xtc = np.ascontiguousarray(xb.transpose(2, 1, 0)).astype(NPBF)  # [65, t, 128]
        in_maps.append({"xt": xtc, **shared})
    return in_maps


def kernel(x, W_ih0, W_hh0, b_ih0, b_hh0, W_ih1, W_hh1, b_ih1, b_hh1, fc_w, fc_b):
    t_steps = T_STEPS
    nc = _get_nc(t_steps)
    in_maps = _prep_inputs(x, W_ih0, W_hh0, b_ih0, b_hh0, W_ih1, W_hh1,
                           b_ih1, b_hh1, fc_w, fc_b, t_steps)
    res = run_bass_kernel_spmd(
        nc, in_maps, core_ids=list(range(NCORES)), trace=TRACE, **TRACE_KW
    )
    kernel.last_results = res
    out = np.concatenate([r["out"][:, 0] for r in res.results]).astype(np.float32)
    return out


# revision 16
# speedup vs baseline: 1.5381x; 1.5381x over previous
"""Trainium2 Bass kernel for a 2-layer LSTM regressor.

Model: x[1024,512,64] -> LSTM(64->256) -> LSTM(256->256) -> tanh(fc(h_last)) -> [1024]

Strategy (data-parallel over batch, 8 cores, 128 rows/core):
  - All matmul inputs bf16; PSUM accumulation fp32.
  - sigmoid folded into tanh: sigmoid(z) = 0.5*(1+tanh(z/2)); the 0.5 input
    prescale is folded into the f/o/i rows of all weight matrices, so ONE
    tanh activation function covers all 4 gate blocks.
  - Gate column order [f o | i g] (PSUM bank0 = f,o; bank1 = i,g) so the
    f-path (u) resolves one bank earlier than the i,g-path (v).
  - State rescaling: store h2 := 2h and C := 2c. Then with t* = tanh blocks:
        u = (tf+1)*C ; v = (ti+1)*tg ; C_new = 0.5*u + v
        tanh_c = tanh(0.5*C_new)     (ACT free scale)
        h2 = (to+1)*tanh_c
    each a single fused scalar_tensor_tensor; h2-consuming weights prescaled
    by 0.5.
  - gates[B=128, 4H] via PSUM with stationary lhsT = transposed state hT;
    h2 transposed on the PE each step (2x 128x128 bf16 transposes/layer).
  - Software-pipelined emission: per-step matmul groups are split into
    "early" contributions (L1 bias, h1T(t-1), x(t+1)) that run during the
    layer-0 elementwise chain, and "late" h0T(t)-dependent contributions,
    avoiding PE FIFO head-of-line blocking behind transposes.
  - Biases: layer-0 bias rides in an appended ones-row of xT (K=65);
    layer-1 bias is a rank-1 matmul.
  - x pre-transposed host-side to [65, T, 128] per core, fully SBUF-resident.
"""

import numpy as np
import ml_dtypes

import concourse.bass as bass
import concourse.mybir as mybir
import concourse.tile as tile
from concourse import bacc
from concourse.bass_utils import run_bass_kernel_spmd

B, T, D, H = 1024, 512, 64, 256
NCORES = 8
BLOC = B // NCORES  # 128
G4 = 4 * H          # 1024

BF16 = mybir.dt.bfloat16
F32 = mybir.dt.float32
AF = mybir.ActivationFunctionType
OP = mybir.AluOpType
NPBF = ml_dtypes.bfloat16

# Module-level knobs (test.py may override before calling kernel()).
T_STEPS = T
TRACE = False
TRACE_KW = {}

# scheduling knobs (sweepable)
G0_SPLIT = False  # gates-0 as two single-bank tiles (bufs=3) vs one 2-bank tile (bufs=2)
G1_SPLIT = True   # same for gates-1; G0_SPLIT and G1_SPLIT can't both be False (PSUM)
LATE_B1_FIRST = False
ACT_F_FIRST = True
U_FIRST = True
SINGLE_COPY = True
HP_OFFSET = 0

_build_cache = {}


def _build(t_steps):
    nc = bacc.Bacc(
        "TRN2",
        target_bir_lowering=False,
        debug=False,
        enable_asserts=False,
        num_devices=NCORES,
    )

    xt = nc.dram_tensor("xt", [D + 1, t_steps, BLOC], BF16, kind="ExternalInput").ap()
    rih0 = nc.dram_tensor("rih0", [D + 1, G4], BF16, kind="ExternalInput").ap()
    rhh0 = nc.dram_tensor("rhh0", [128, 2 * G4], BF16, kind="ExternalInput").ap()
    rih1 = nc.dram_tensor("rih1", [128, 2 * G4], BF16, kind="ExternalInput").ap()
    rhh1 = nc.dram_tensor("rhh1", [128, 2 * G4], BF16, kind="ExternalInput").ap()
    rb1 = nc.dram_tensor("rb1", [1, G4], BF16, kind="ExternalInput").ap()
    onesd = nc.dram_tensor("ones", [1, BLOC], BF16, kind="ExternalInput").ap()
    identd = nc.dram_tensor("ident", [128, 128], BF16, kind="ExternalInput").ap()
    fcwd = nc.dram_tensor("fcw", [128, 2], BF16, kind="ExternalInput").ap()
    fcbd = nc.dram_tensor("fcb", [128, 1], F32, kind="ExternalInput").ap()
    outd = nc.dram_tensor("out", [BLOC, 1], F32, kind="ExternalOutput").ap()

    with tile.TileContext(nc) as tc:
        with (
            tc.tile_pool(name="const", bufs=1) as cp,
            tc.tile_pool(name="tg", bufs=2) as tgp,
            tc.tile_pool(name="uvp", bufs=2) as uvp,
            tc.tile_pool(name="cst", bufs=2) as cstp,
            tc.tile_pool(name="hT", bufs=3) as hTp,
            tc.tile_pool(name="g0psum", bufs=3 if G0_SPLIT else 2,
                         space="PSUM") as g0p,
            tc.tile_pool(name="g1psum", bufs=3 if G1_SPLIT else 2,
                         space="PSUM") as g1p,
            tc.tile_pool(name="hpsum", bufs=1, space="PSUM") as hp,
        ):
            # ---- constants / inputs resident in SBUF ----
            xt_sb = cp.tile([D + 1, t_steps, BLOC], BF16)
            chunk = 64
            for c0 in range(0, t_steps, chunk):
                c1 = min(c0 + chunk, t_steps)
                nc.sync.dma_start(out=xt_sb[:, c0:c1, :], in_=xt[:, c0:c1, :])
            rih0_sb = cp.tile([D + 1, G4], BF16)
            nc.sync.dma_start(out=rih0_sb, in_=rih0)
            rhh0_sb = cp.tile([128, 2 * G4], BF16)
            nc.sync.dma_start(out=rhh0_sb, in_=rhh0)
            rih1_sb = cp.tile([128, 2 * G4], BF16)
            nc.sync.dma_start(out=rih1_sb, in_=rih1)
            rhh1_sb = cp.tile([128, 2 * G4], BF16)
            nc.sync.dma_start(out=rhh1_sb, in_=rhh1)
            rb1_sb = cp.tile([1, G4], BF16)
            nc.sync.dma_start(out=rb1_sb, in_=rb1)
            ones_sb = cp.tile([1, BLOC], BF16)
            nc.sync.dma_start(out=ones_sb, in_=onesd)
            ident_sb = cp.tile([128, 128], BF16)
            nc.sync.dma_start(out=ident_sb, in_=identd)
            fcw_sb = cp.tile([128, 2], BF16)
            nc.sync.dma_start(out=fcw_sb, in_=fcwd)
            fcb_sb = cp.tile([128, 1], F32)
            nc.sync.dma_start(out=fcb_sb, in_=fcbd)

            # ---- initial state ----
            h0T = hTp.tile([128, 2 * BLOC], BF16, tag="h0T")
            h1T = hTp.tile([128, 2 * BLOC], BF16, tag="h1T")
            C0 = cstp.tile([128, H], F32, tag="C0")
            C1 = cstp.tile([128, H], F32, tag="C1")
            nc.vector.memset(h0T, 0.0)
            nc.vector.memset(h1T, 0.0)
            nc.vector.memset(C0, 0.0)
            nc.vector.memset(C1, 0.0)

            def bank_tiles(pool, tag, split):
                if split:
                    ga = pool.tile([128, 512], F32, tag=tag, name=f"{tag}a")
                    gb = pool.tile([128, 512], F32, tag=tag, name=f"{tag}b")
                    return [ga, gb]
                G = pool.tile([128, G4], F32, tag=tag, name=f"{tag}w")
                return [G[:, 0:512], G[:, 512:1024]]

            def g0_early(t):
                Gb = bank_tiles(g0p, "G0", G0_SPLIT)
                for nb in range(2):
                    n0, n1 = nb * 512, (nb + 1) * 512
                    nc.tensor.matmul(Gb[nb], xt_sb[:, t, :], rih0_sb[:, n0:n1],
                                     start=True, stop=False)
                return Gb

            def g0_late(Gb, h0T_cur):
                for nb in ((1, 0) if LATE_B1_FIRST else (0, 1)):
                    n0, n1 = nb * 512, (nb + 1) * 512
                    for k in range(2):
                        nc.tensor.matmul(
                            Gb[nb], h0T_cur[:, k * BLOC:(k + 1) * BLOC],
                            rhh0_sb[:, k * G4 + n0:k * G4 + n1],
                            start=False, stop=k == 1)

            def g1_early(h1T_prev):
                Gb = bank_tiles(g1p, "G1", G1_SPLIT)
                for nb in range(2):
                    n0, n1 = nb * 512, (nb + 1) * 512
                    nc.tensor.matmul(Gb[nb], ones_sb, rb1_sb[:, n0:n1],
                                     start=True, stop=False)
                    for k in range(2):
                        nc.tensor.matmul(
                            Gb[nb], h1T_prev[:, k * BLOC:(k + 1) * BLOC],
                            rhh1_sb[:, k * G4 + n0:k * G4 + n1],
                            start=False, stop=False)
                return Gb

            def g1_late(Gb, h0T_cur):
                for nb in range(2):
                    n0, n1 = nb * 512, (nb + 1) * 512
                    for k in range(2):
                        nc.tensor.matmul(
                            Gb[nb], h0T_cur[:, k * BLOC:(k + 1) * BLOC],
                            rih1_sb[:, k * G4 + n0:k * G4 + n1],
                            start=False, stop=k == 1)

            def cell_front(layer, G, C_prev, split_act):
                """Gate tanhs + cell-state update. Returns (tg, C_new).
                Gate blocks: ti=[0:H], tg=[H:2H], tf=[2H:3H], to=[3H:4H]."""
                tg = tgp.tile([128, G4], BF16, tag=f"tg{layer}", name=f"tg{layer}")
                Gb0, Gb1 = G[0], G[1]
                if split_act:  # layer 0
                    if ACT_F_FIRST:
                        nc.scalar.activation(tg[:, 512:768], Gb1[:, 0:256], AF.Tanh)
                        nc.scalar.activation(tg[:, 0:512], Gb0[:, 0:512], AF.Tanh)
                    else:
                        nc.scalar.activation(tg[:, 0:512], Gb0[:, 0:512], AF.Tanh)
                        nc.scalar.activation(tg[:, 512:768], Gb1[:, 0:256], AF.Tanh)
                    nc.scalar.activation(tg[:, 768:1024], Gb1[:, 256:512], AF.Tanh)
                else:          # layer 1
                    nc.scalar.activation(tg[:, 0:512], Gb0, AF.Tanh)
                    nc.scalar.activation(tg[:, 512:1024], Gb1, AF.Tanh)
                u = uvp.tile([128, H], F32, tag=f"u{layer}", name=f"u{layer}")
                v = uvp.tile([128, H], BF16, tag=f"v{layer}", name=f"v{layer}")
                def emit_u():
                    nc.vector.scalar_tensor_tensor(
                        u, tg[:, 2 * H:3 * H], 1.0, C_prev, op0=OP.add, op1=OP.mult)
                def emit_v():
                    nc.vector.scalar_tensor_tensor(
                        v, tg[:, 0:H], 1.0, tg[:, H:2 * H], op0=OP.add, op1=OP.mult)
                if U_FIRST:
                    emit_u(); emit_v()
                else:
                    emit_v(); emit_u()
                C_new = cstp.tile([128, H], F32, tag=f"C{layer}", name=f"C{layer}")
                nc.vector.scalar_tensor_tensor(
                    C_new, u, 0.5, v, op0=OP.mult, op1=OP.add)
                return tg, C_new

            def cell_back(layer, tg, C_new):
                """tanh(c), output gate, transpose. Returns hT_new."""
                tc_ = uvp.tile([128, H], BF16, tag=f"tc{layer}", name=f"tc{layer}")
                nc.scalar.activation(tc_, C_new, AF.Tanh, scale=0.5)
                h2 = uvp.tile([128, H], BF16, tag=f"h2{layer}", name=f"h2{layer}")
                nc.vector.scalar_tensor_tensor(
                    h2, tg[:, 3 * H:4 * H], 1.0, tc_, op0=OP.add, op1=OP.mult)
                hps = hp.tile([128, 2 * BLOC], BF16, tag="hps", name="hps")
                nc.tensor.transpose(hps[:, 0:BLOC], h2[:, 0:BLOC], ident_sb)
                nc.tensor.transpose(hps[:, BLOC:2 * BLOC], h2[:, BLOC:2 * BLOC],
                                    ident_sb)
                hT_new = hTp.tile([128, 2 * BLOC], BF16, tag=f"h{layer}T",
                                  name=f"h{layer}T")
                if SINGLE_COPY:
                    nc.vector.tensor_copy(hT_new, hps)
                else:
                    nc.vector.tensor_copy(hT_new[:, 0:BLOC], hps[:, 0:BLOC])
                    nc.vector.tensor_copy(hT_new[:, BLOC:2 * BLOC],
                                          hps[:, BLOC:2 * BLOC])
                return hT_new

            # ---- software-pipelined main loop ----
            # L1 runs one step behind L0 and its cell is further split so its
            # tanh_c never head-of-line-blocks the next L0 gate-tanhs on ACT.
            G0 = g0_early(0)
            g0_late(G0, h0T)
            G1 = None
            for t in range(t_steps):
                G0n = g0_early(t + 1) if t + 1 < t_steps else None
                tg0, C0 = cell_front(0, G0, C0, split_act=True)
                h0T = cell_back(0, tg0, C0)
                if G0n is not None:
                    g0_late(G0n, h0T)
                if G1 is not None:
                    tg1, C1 = cell_front(1, G1, C1, split_act=False)
                    h1T = cell_back(1, tg1, C1)
                G1 = g1_early(h1T)
                g1_late(G1, h0T)
                G0 = G0n
            tg1, C1 = cell_front(1, G1, C1, split_act=False)
            h1T = cell_back(1, tg1, C1)
            # ---- final FC head: out = tanh(h1 @ fc_w.T + fc_b) ----
            Gfc = g1p.tile([128, 512] if G1_SPLIT else [128, G4], F32,
                           tag="G1", name="Gfc")
            nc.tensor.matmul(Gfc[:, 0:1], h1T[:, 0:BLOC], fcw_sb[:, 0:1],
                             start=True, stop=False)
            nc.tensor.matmul(Gfc[:, 0:1], h1T[:, BLOC:2 * BLOC], fcw_sb[:, 1:2],
                             start=False, stop=True)
            out_sb = cp.tile([128, 1], F32)
            nc.scalar.activation(out_sb, Gfc[:, 0:1], AF.Tanh, bias=fcb_sb)
            nc.sync.dma_start(out=outd, in_=out_sb)

    nc.compile()
    return nc


def _get_nc(t_steps):
    if t_steps not in _build_cache:
        _build_cache[t_steps] = _build(t_steps)
    return _build_cache[t_steps]


def _prep_inputs(x, W_ih0, W_hh0, b_ih0, b_hh0, W_ih1, W_hh1, b_ih1, b_hh1,
                 fc_w, fc_b, t_steps):
    f32 = np.float32
    # gate column permutation [i f g o] -> [i g f o], with 0.5 prescale on
    # the sigmoid gates (i, f, o) for the tanh trick
    perm = np.concatenate([np.arange(0, H), np.arange(2 * H, 3 * H),
                           np.arange(H, 2 * H), np.arange(3 * H, 4 * H)])
    gs = np.ones(G4, f32)
    gs[0:H] = 0.5
    gs[2 * H:] = 0.5

    def gp(a):  # permute gate columns of [K, 4H] and apply prescale
        return a[:, perm] * gs

    def chunks2(a):  # [256, G4] -> [128, 2*G4], chunk k at cols [k*G4:(k+1)*G4]
        return np.concatenate([a[0:128, :], a[128:256, :]], axis=1)

    rih0 = gp(np.concatenate(
        [W_ih0.T.astype(f32), (b_ih0 + b_hh0)[None, :].astype(f32)], 0))
    rhh0 = chunks2(gp(W_hh0.T.astype(f32) * 0.5))
    rih1 = chunks2(gp(W_ih1.T.astype(f32) * 0.5))
    rhh1 = chunks2(gp(W_hh1.T.astype(f32) * 0.5))
    rb1 = gp((b_ih1 + b_hh1).astype(f32)[None, :])
    fcw = np.empty((128, 2), f32)
    fcw[:, 0] = fc_w[0, 0:128] * 0.5
    fcw[:, 1] = fc_w[0, 128:256] * 0.5

    shared = {
        "rih0": rih0.astype(NPBF),
        "rhh0": rhh0.astype(NPBF),
        "rih1": rih1.astype(NPBF),
        "rhh1": rhh1.astype(NPBF),
        "rb1": rb1.astype(NPBF),
        "ones": np.ones((1, BLOC), NPBF),
        "ident": np.eye(128, dtype=NPBF),
        "fcw": fcw.astype(NPBF),
        "fcb": np.full((128, 1), np.float32(fc_b[0]), f32),
    }

    in_maps = []
    for c in range(NCORES):
        xc = np.asarray(x[c * BLOC:(c + 1) * BLOC, :t_steps, :], f32)
        xb = np.concatenate([xc, np.ones((BLOC, t_steps, 1), f32)], 2)
        xtc = np.ascontiguousarray(xb.transpose(2, 1, 0)).astype(NPBF)
        in_maps.append({"xt": xtc, **shared})
    return in_maps


def kernel(x, W_ih0, W_hh0, b_ih0, b_hh0, W_ih1, W_hh1, b_ih1, b_hh1, fc_w, fc_b):
    t_steps = T_STEPS
    nc = _get_nc(t_steps)
    in_maps = _prep_inputs(x, W_ih0, W_hh0, b_ih0, b_hh0, W_ih1, W_hh1,
                           b_ih1, b_hh1, fc_w, fc_b, t_steps)
    res = run_bass_kernel_spmd(
        nc, in_maps, core_ids=list(range(NCORES)), trace=TRACE, **TRACE_KW
    )
    kernel.last_results = res
    out = np.concatenate([r["out"][:, 0] for r in res.results]).astype(np.float32)
    return out
